# revision 1
# baseline (speedup 1.0000x reference)
"""Trainium2 Bass kernel for nn_CumulativeProbingDense.

Computation (see reference):
    h      = sum_l softmax(mixing_weights)[l] * x[:, l] * gamma   # [B, S, F]
    h1     = relu(h @ W1.T + b1)                                  # [B, S, H]
    h2     = relu(h1 @ W2.T + b2)                                 # [B, S, H]
    pooled = (h2 * mask).sum(S) / lengths                         # [B, H]
    logits = pooled @ Wl.T + bl                                   # [B, NL]

Sharding: pure data parallel over batch, 2 samples per core on 8 cores.
The dominant cost is streaming x (654 MB fp32) from HBM once.

Device strategy per core:
  - layer mix on the TensorE: PSUM-accumulated matmuls with a scaled
    identity as the stationary operand and x tiles (natural [token, feat]
    layout) as the moving operand -> h tile [128 t, 768 f]
  - PE transpose h tiles into hT [feat partitions, token free]
  - MLP matmuls with W1T/W2T chunks stationary, relu+bias on ScalarE
  - masked mean-pool with one fused DVE tensor_tensor_reduce against a
    host-prebuilt mask/length tile, then a tiny matmul for the logits
"""

import numpy as np

import concourse.bass as bass
import concourse.tile as tile
from concourse import mybir
from concourse.bass_utils import run_bass_kernel_spmd
from contextlib import ExitStack

F32 = mybir.dt.float32
F32R = mybir.dt.float32r

N_CORES = 8
B, L, S, F = 16, 13, 1024, 768
H, NL = 256, 7
B_LOC = B // N_CORES          # samples per core
P = 128                       # SBUF partitions
TT = S // P                   # token tiles per sample
FC = F // P                   # feature chunks of 128
HC = H // P                   # hidden chunks of 128

# matmul input dtype: float32r streams at 1 cycle/row (moving dim >= 256)
# vs plain float32's 4 cycles/row. fp32r rounds the operands (TF32-like),
# measured end-to-end rel err vs the fp32 reference on hardware: ~1.8e-4.
MM_DT = F32R


def _split_excess_waits(nc, max_waits=1):
    """walrus (CoreV3) rejects instructions carrying more than a couple of
    sync waits (e.g. the TileContext exit drain). Hoist excess waits onto
    standalone NoOps inserted before the offending instruction."""
    n_fixed = 0
    for f in nc.m.functions:
        for bb in f.blocks:
            out, changed = [], False
            for inst in bb.instructions:
                si = getattr(inst, "sync_info", None)
                if si is not None and len(si.on_wait) > max_waits:
                    waits = list(si.on_wait)
                    for j, w in enumerate(waits[max_waits:]):
                        out.append(mybir.InstNoOp(
                            name=f"{inst.name}-wsplit{j}",
                            engine=inst.engine, ins=[], outs=[],
                            sync_info=mybir.SyncInfo(on_wait=[w], on_update=[]),
                        ))
                    inst.sync_info = mybir.SyncInfo(
                        on_wait=waits[:max_waits], on_update=list(si.on_update))
                    changed = True
                    n_fixed += 1
                out.append(inst)
            if changed:
                bb.instructions = out
    return n_fixed


def _r(ap):
    return ap


def build_program(n_layers: int, split_waits: bool = True, repeat: int = 1,
                  batched_dma: bool = True,
                  hw_loop_repeat: int | None = None,
                  mix_dve_layers: int = 0,
                  x_bufs: int = 2, ht_bufs: int = 2,
                  dma_pieces: int = 2) -> bass.Bass:
    # mix_dve_layers: how many of the trailing layers are accumulated on the
    # DVE (axpy) instead of the TensorE, to balance PE vs DVE occupancy.
    n_pe_layers = n_layers - mix_dve_layers
    assert n_pe_layers >= 1
    nc = bass.Bass("TRN2", target_bir_lowering=False, debug=False, num_devices=1)

    x_d = nc.dram_tensor("x", [B_LOC, L, S, F], F32R, kind="ExternalInput").ap()
    seye_d = nc.dram_tensor("seye", [P, L * P], F32R, kind="ExternalInput").ap()
    ident_d = nc.dram_tensor("ident", [P, P], F32, kind="ExternalInput").ap()
    w1t_d = nc.dram_tensor("w1t", [P, FC * H], F32R, kind="ExternalInput").ap()
    w2t_d = nc.dram_tensor("w2t", [P, HC * H], F32R, kind="ExternalInput").ap()
    wlt_d = nc.dram_tensor("wlt", [P, HC * NL], F32, kind="ExternalInput").ap()
    b1_d = nc.dram_tensor("b1", [P, HC], F32, kind="ExternalInput").ap()
    b2_d = nc.dram_tensor("b2", [P, HC], F32, kind="ExternalInput").ap()
    bl_d = nc.dram_tensor("bl", [NL, 1], F32, kind="ExternalInput").ap()
    msk_d = nc.dram_tensor("msk", [P, B_LOC * S], F32, kind="ExternalInput").ap()
    svec_d = nc.dram_tensor("svec", [P, L], F32, kind="ExternalInput").ap()
    out_d = nc.dram_tensor("out", [B_LOC, NL], F32, kind="ExternalOutput").ap()

    with TileKernel(nc) as (tc, ctx):
        const = ctx.enter_context(tc.tile_pool(name="const", bufs=1))
        xpool = ctx.enter_context(tc.tile_pool(name="x", bufs=x_bufs))
        hpool = ctx.enter_context(tc.tile_pool(name="h", bufs=3))
        htpool = ctx.enter_context(tc.tile_pool(name="ht", bufs=ht_bufs))
        apool = ctx.enter_context(tc.tile_pool(name="acts", bufs=1 if batched_dma else 2))
        spool = ctx.enter_context(tc.tile_pool(name="small", bufs=2 if batched_dma else 4))
        pmix0 = ctx.enter_context(tc.tile_pool(name="pmix0", bufs=2, space="PSUM"))
        pmix1 = ctx.enter_context(tc.tile_pool(name="pmix1", bufs=2, space="PSUM"))
        ptr = ctx.enter_context(tc.tile_pool(name="ptr", bufs=2, space="PSUM"))
        pout = ctx.enter_context(tc.tile_pool(name="pout", bufs=2, space="PSUM"))

        # ---- constants into SBUF via SWDGE (gpsimd), keeping both HWDGE
        # rings free for the x stream ----
        seye = const.tile([P, L * P], F32R)
        nc.gpsimd.dma_start(seye[:], seye_d[:])
        ident = const.tile([P, P], F32)
        nc.gpsimd.dma_start(ident[:], ident_d[:])
        w1t = const.tile([P, FC * H], F32R)
        nc.gpsimd.dma_start(w1t[:], w1t_d[:])
        w2t = const.tile([P, HC * H], F32R)
        nc.gpsimd.dma_start(w2t[:], w2t_d[:])
        wlt = const.tile([P, HC * NL], F32)
        nc.gpsimd.dma_start(wlt[:], wlt_d[:])
        b1 = const.tile([P, HC], F32)
        nc.gpsimd.dma_start(b1[:], b1_d[:])
        b2 = const.tile([P, HC], F32)
        nc.gpsimd.dma_start(b2[:], b2_d[:])
        bl = const.tile([NL, 1], F32)
        nc.gpsimd.dma_start(bl[:], bl_d[:])
        msk = const.tile([P, B_LOC * S], F32)
        nc.gpsimd.dma_start(msk[:], msk_d[:])
        svec = const.tile([P, L], F32)
        nc.gpsimd.dma_start(svec[:], svec_d[:])

        logits = const.tile([NL, B_LOC], F32)

        CW = 256                # token width of one streamed MLP chunk
        NCH = S // CW           # chunks per sample

        def mlp_chunk(b, n, hT, h1, h2, pooled4):
            """mm1 + mm2 + relus + chunk pooling for token chunk n."""
            for m in range(HC):
                o1 = pout.tile([P, CW], F32, tag="po")
                for k in range(FC):
                    lhs = w1t[:, k * H + m * P: k * H + (m + 1) * P]
                    rhs = hT[:, k * S + n * CW: k * S + (n + 1) * CW]
                    nc.tensor.matmul(o1[:], lhs, rhs,
                                     start=(k == 0), stop=(k == FC - 1))
                nc.scalar.activation(
                    h1[:, m * S + n * CW: m * S + (n + 1) * CW], o1[:],
                    mybir.ActivationFunctionType.Relu,
                    bias=b1[:, m:m + 1], scale=1.0)
            for m in range(HC):
                o2 = pout.tile([P, CW], F32, tag="po")
                for k in range(HC):
                    lhs = w2t[:, k * H + m * P: k * H + (m + 1) * P]
                    rhs = h1[:, k * S + n * CW: k * S + (n + 1) * CW]
                    nc.tensor.matmul(o2[:], lhs, rhs,
                                     start=(k == 0), stop=(k == HC - 1))
                nc.scalar.activation(
                    h2[:, m * S + n * CW: m * S + (n + 1) * CW], o2[:],
                    mybir.ActivationFunctionType.Relu,
                    bias=b2[:, m:m + 1], scale=1.0)
                # masked partial pool of this chunk -> pooled4[m][:, n]
                junk = spool.tile([P, CW], F32, tag="junk")
                nc.vector.scalar_tensor_tensor(
                    out=junk[:], in0=h2[:, m * S + n * CW: m * S + (n + 1) * CW],
                    scalar=1.0, in1=msk[:, b * S + n * CW: b * S + (n + 1) * CW],
                    op0=mybir.AluOpType.bypass, op1=mybir.AluOpType.mult,
                    accum_out=pooled4[m][:, n:n + 1])

        def _body(_iv=None):
          for b in range(B_LOC):
            # hT[fc block of 1024 cols] = transposed mixed features
            hT = htpool.tile([P, FC * S], F32R, tag="hT")
            h1 = apool.tile([P, HC * S], F32R, tag="h1")
            h2 = apool.tile([P, HC * S], F32, tag="h2")
            pooled4 = [spool.tile([P, NCH], F32, tag=f"pool{m}", name=f"pool{m}")
                       for m in range(HC)]

            for ti in range(TT):
                pm0 = pmix0.tile([P, 512], F32, tag="pm0")
                pm1 = pmix1.tile([P, F - 512], F32, tag="pm1")
                # All x DMAs ride the SP HWDGE ring (SP has no other work,
                # so triggers never queue behind compute). Two pieces per
                # token tile so the mix can start on the first piece while
                # the second is still in flight.
                xt13 = xpool.tile([P, n_layers, F], F32R, tag="xt")
                if isinstance(dma_pieces, (list, tuple)):
                    bounds = sorted({min(bd, n_layers) for bd in dma_pieces})
                else:
                    bounds = [round(i * n_layers / dma_pieces)
                              for i in range(dma_pieces + 1)]
                for lo, hi in zip(bounds[:-1], bounds[1:]):
                    src = x_d[b, lo:hi, ti * P:(ti + 1) * P, :] \
                        .rearrange("l t f -> t l f")
                    nc.sync.dma_start(xt13[:, lo:hi], src)
                accd = None
                for l in range(n_layers):
                    xrow = xt13[:, l]
                    if l < n_pe_layers:
                        se = seye[:, l * P:(l + 1) * P]
                        st, sp = (l == 0), (l == n_pe_layers - 1)
                        nc.tensor.matmul(pm0[:], se, xrow[:, 0:512],
                                         start=st, stop=sp)
                        nc.tensor.matmul(pm1[:], se, xrow[:, 512:F],
                                         start=st, stop=sp)
                    else:
                        xf = xrow.bitcast(F32)
                        sc = svec[:, l:l + 1]
                        if accd is None:
                            accd = hpool.tile([P, F], F32, tag="accd")
                            nc.vector.tensor_scalar_mul(accd[:], xf, sc)
                        else:
                            nc.vector.scalar_tensor_tensor(
                                accd[:], xf, sc, accd[:],
                                op0=mybir.AluOpType.mult, op1=mybir.AluOpType.add)
                # PSUM (+ DVE partial) -> SBUF mixed tile
                h = hpool.tile([P, F], F32, tag="h")
                if accd is None:
                    nc.scalar.copy(h[:, 0:512], pm0[:])
                    nc.scalar.copy(h[:, 512:F], pm1[:])
                else:
                    nc.vector.scalar_tensor_tensor(
                        h[:, 0:512], pm0[:], 1.0, accd[:, 0:512],
                        op0=mybir.AluOpType.bypass, op1=mybir.AluOpType.add)
                    nc.vector.scalar_tensor_tensor(
                        h[:, 512:F], pm1[:], 1.0, accd[:, 512:F],
                        op0=mybir.AluOpType.bypass, op1=mybir.AluOpType.add)
                # transpose 128x128 blocks into hT
                for fc in range(FC):
                    pt = ptr.tile([P, P], F32, tag="pt")
                    nc.tensor.transpose(pt[:], h[:, fc * P:(fc + 1) * P], ident[:])
                    dst = hT[:, fc * S + ti * P: fc * S + (ti + 1) * P]
                    if fc % 2 == 0 or accd is not None:
                        nc.scalar.copy(dst, pt[:])
                    else:
                        nc.vector.tensor_copy(dst, pt[:])
                # stream the MLP over finished 512-token chunks so only the
                # last chunk's matmuls remain after the final DMA
                if (ti + 1) % (TT // NCH) == 0:
                    mlp_chunk(b, (ti + 1) // (TT // NCH) - 1, hT, h1, h2, pooled4)

            # ---- combine partial pools + logits ----
            plog = pout.tile([NL, 1], F32, tag="po")
            for m in range(HC):
                pooled = spool.tile([P, 1], F32, tag="pooled")
                nc.vector.tensor_reduce(pooled[:], pooled4[m][:],
                                        mybir.AxisListType.X,
                                        mybir.AluOpType.add)
                nc.tensor.matmul(plog[:], wlt[:, m * NL:(m + 1) * NL],
                                 pooled[:],
                                 start=(m == 0), stop=(m == HC - 1))
            nc.vector.tensor_tensor(logits[:, b:b + 1], plog[:], bl[:],
                                    mybir.AluOpType.add)

        if hw_loop_repeat is not None and hw_loop_repeat > 1:
            with tc.For_i(0, hw_loop_repeat, 1) as _i:
                _body(_i)
        else:
            for _rep in range(repeat):
                _body()

        nc.sync.dma_start(out_d.rearrange("o f -> f o"), logits[:])

    if split_waits:
        _split_excess_waits(nc, max_waits=1)
    return nc


class TileKernel:
    """TileContext + ExitStack in one `with`."""

    def __init__(self, nc):
        self.tc = tile.TileContext(nc)
        self.ctx = ExitStack()

    def __enter__(self):
        tc = self.tc.__enter__()
        self.ctx.__enter__()
        return tc, self.ctx

    def __exit__(self, *exc):
        self.ctx.__exit__(*exc)
        return self.tc.__exit__(*exc)


_PROGRAM_CACHE: dict[int, bass.Bass] = {}


def _get_program(n_layers: int) -> bass.Bass:
    if n_layers not in _PROGRAM_CACHE:
        _PROGRAM_CACHE[n_layers] = build_program(n_layers)
    return _PROGRAM_CACHE[n_layers]


def _softmax32(v: np.ndarray) -> np.ndarray:
    v = v.astype(np.float32)
    e = np.exp(v - v.max())
    return (e / e.sum()).astype(np.float32)


def _prep_in_maps(inputs: dict) -> list[dict]:
    x = np.asarray(inputs["x"])
    lengths = np.asarray(inputs["lengths"])

    # host-side prep of the small replicated operands
    s = (_softmax32(np.asarray(inputs["mixing_weights"]))
         * np.float32(np.asarray(inputs["gamma"]).reshape(-1)[0]))
    seye = np.zeros((P, L * P), np.float32)
    for l in range(L):
        seye[:, l * P:(l + 1) * P] = np.eye(P, dtype=np.float32) * s[l]
    ident = np.eye(P, dtype=np.float32)

    W1 = np.asarray(inputs["W1"], np.float32)  # [H, F]
    W2 = np.asarray(inputs["W2"], np.float32)  # [H, H]
    Wl = np.asarray(inputs["Wl"], np.float32)  # [NL, H]
    w1t = np.ascontiguousarray(
        W1.T.reshape(FC, P, H).transpose(1, 0, 2).reshape(P, FC * H))
    w2t = np.ascontiguousarray(
        W2.T.reshape(HC, P, H).transpose(1, 0, 2).reshape(P, HC * H))
    wlt = np.ascontiguousarray(
        Wl.T.reshape(HC, P, NL).transpose(1, 0, 2).reshape(P, HC * NL))
    b1p = np.ascontiguousarray(np.asarray(inputs["b1"], np.float32).reshape(HC, P).T)
    b2p = np.ascontiguousarray(np.asarray(inputs["b2"], np.float32).reshape(HC, P).T)
    blp = np.asarray(inputs["bl"], np.float32).reshape(NL, 1)

    in_maps = []
    for c in range(N_CORES):
        sl = slice(c * B_LOC, (c + 1) * B_LOC)
        lens = lengths[sl].astype(np.float32)
        msk = np.zeros((P, B_LOC * S), np.float32)
        for b in range(B_LOC):
            msk[:, b * S:(b + 1) * S] = (
                (np.arange(S, dtype=np.float32) < lens[b]) / lens[b])[None, :]
        in_maps.append({
            "x": np.ascontiguousarray(x[sl]),
            "seye": seye, "ident": ident,
            "w1t": w1t, "w2t": w2t, "wlt": wlt,
            "b1": b1p, "b2": b2p, "bl": blp,
            "msk": msk,
            "svec": np.ascontiguousarray(np.tile(s, (P, 1))),
        })
    return in_maps


def kernel(x, lengths, layer, gamma, mixing_weights, W1, b1, W2, b2, Wl, bl):
    n_layers = int(np.asarray(layer)) + 1
    assert 1 <= n_layers <= L

    nc = _get_program(n_layers)
    in_maps = _prep_in_maps(dict(
        x=x, lengths=lengths, gamma=gamma, mixing_weights=mixing_weights,
        W1=W1, b1=b1, W2=W2, b2=b2, Wl=Wl, bl=bl))

    res = run_bass_kernel_spmd(nc, in_maps, list(range(N_CORES)))
    return np.concatenate([res.results[c]["out"] for c in range(N_CORES)], axis=0)



# revision 4
# speedup vs baseline: 1.8757x; 1.8757x over previous
"""Trainium2 Bass kernel for nn_CumulativeProbingDense.

Computation (see reference):
    h      = sum_l softmax(mixing_weights)[l] * x[:, l] * gamma   # [B, S, F]
    h1     = relu(h @ W1.T + b1)                                  # [B, S, H]
    h2     = relu(h1 @ W2.T + b2)                                 # [B, S, H]
    pooled = (h2 * mask).sum(S) / lengths                         # [B, H]
    logits = pooled @ Wl.T + bl                                   # [B, NL]

Sharding: tokens masked out by `lengths` (t >= lengths[b]) cannot affect the
output, so only the sum(lengths) valid tokens are processed.  The host packs
the valid (sample, token) pairs of ALL samples into one stream, split evenly
across the 8 cores (token-balanced data parallel).  Each core streams its
packed x slice ([t_tiles*128 tokens, L, F], token-major so DMA runs are
contiguous), computes the layer mix + MLP, and mask-matmul-pools per sample.
Host combines the per-core pooled partials and applies the tiny final linear.

Device pipeline per 128-token tile:
  - layer mix on TensorE: PSUM-accumulated matmuls with scaled-identity
    stationary; optionally a few trailing layers accumulate on the DVE
  - PE-transpose mixed tile into hT [feat part, token free]
  - mm1 (W1T stationary) -> relu+b1 on ScalarE -> h1 [hid part, token free]
  - mm2 with h1 chunks as STATIONARY and W2T moving -> h2 [token part, hid
    free]; b2 enters the same PSUM group as a ones x (b2/128) matmul
  - pooling: matmul with the per-sample 0/1 mask [token, 16] stationary,
    PSUM-accumulated over all tiles -> pooled [16, 256] per core
"""

import numpy as np

import concourse.bass as bass
import concourse.tile as tile
from concourse import mybir
from concourse.bass_utils import run_bass_kernel_spmd
from contextlib import ExitStack

F32 = mybir.dt.float32
F32R = mybir.dt.float32r

N_CORES = 8
B, L, S, F = 16, 13, 1024, 768
H, NL = 256, 7
P = 128                       # SBUF partitions
FC = F // P                   # feature chunks of 128
HC = H // P                   # hidden chunks of 128
CWMAX = 2 * P                 # token width of one MLP chunk

# matmul input dtype: float32r streams at 1 cycle/row (moving dim >= 256)
# vs plain float32's 4 cycles/row. fp32r rounds the operands (TF32-like).
MM_DT = F32R


def _split_excess_waits(nc, max_waits=1):
    """walrus (CoreV3) rejects instructions carrying more than a couple of
    sync waits (e.g. the TileContext exit drain). Hoist excess waits onto
    standalone NoOps inserted before the offending instruction."""
    n_fixed = 0
    for f in nc.m.functions:
        for bb in f.blocks:
            out, changed = [], False
            for inst in bb.instructions:
                si = getattr(inst, "sync_info", None)
                if si is not None and len(si.on_wait) > max_waits:
                    waits = list(si.on_wait)
                    for j, w in enumerate(waits[max_waits:]):
                        out.append(mybir.InstNoOp(
                            name=f"{inst.name}-wsplit{j}",
                            engine=inst.engine, ins=[], outs=[],
                            sync_info=mybir.SyncInfo(on_wait=[w], on_update=[]),
                        ))
                    inst.sync_info = mybir.SyncInfo(
                        on_wait=waits[:max_waits], on_update=list(si.on_update))
                    changed = True
                    n_fixed += 1
                out.append(inst)
            if changed:
                bb.instructions = out
    return n_fixed


def _plan_packing(lengths):
    """Token-balanced packing of all valid (sample, token) pairs onto cores.

    Returns (t_tiles, b_idx [n_cores, cap], t_idx [n_cores, cap],
    valid [n_cores, cap]) with cap = t_tiles*128 slots per core; pad slots
    point at (0, 0) with valid=0."""
    lengths = np.asarray(lengths).astype(np.int64)
    total = int(lengths.sum())
    t_tiles = max(1, -(-total // (N_CORES * P)))
    cap = t_tiles * P
    bs = np.repeat(np.arange(B, dtype=np.int64), lengths)
    ts = np.concatenate([np.arange(n, dtype=np.int64) for n in lengths])
    pad = N_CORES * cap - total
    bs = np.concatenate([bs, np.zeros(pad, np.int64)])
    ts = np.concatenate([ts, np.zeros(pad, np.int64)])
    val = np.concatenate([np.ones(total, np.float32), np.zeros(pad, np.float32)])
    return (t_tiles, bs.reshape(N_CORES, cap), ts.reshape(N_CORES, cap),
            val.reshape(N_CORES, cap))


def build_program(n_layers: int, t_tiles: int, split_waits: bool = True,
                  hw_loop_repeat: int | None = None,
                  mix_dve_layers: int = 3,
                  x_bufs: int = 3,
                  dma_pieces: int = 2) -> bass.Bass:
    # mix_dve_layers: trailing layers accumulated on the DVE (axpy) instead
    # of the TensorE, to balance PE vs DVE occupancy.
    n_pe_layers = n_layers - min(mix_dve_layers, n_layers - 1)
    cap = t_tiles * P
    nc = bass.Bass("TRN2", target_bir_lowering=False, debug=False, num_devices=1)

    xp_d = nc.dram_tensor("xp", [cap, n_layers * F], F32R, kind="ExternalInput").ap()
    seye_d = nc.dram_tensor("seye", [P, n_layers * P], F32R, kind="ExternalInput").ap()
    svec_d = nc.dram_tensor("svec", [P, n_layers], F32, kind="ExternalInput").ap()
    ident_d = nc.dram_tensor("ident", [P, P], F32, kind="ExternalInput").ap()
    w1t_d = nc.dram_tensor("w1t", [P, FC * H], F32R, kind="ExternalInput").ap()
    w2t_d = nc.dram_tensor("w2t", [P, HC * H], F32R, kind="ExternalInput").ap()
    b1_d = nc.dram_tensor("b1", [P, HC], F32, kind="ExternalInput").ap()
    b2rep_d = nc.dram_tensor("b2rep", [P, H], F32R, kind="ExternalInput").ap()
    ones_d = nc.dram_tensor("ones", [P, P], F32R, kind="ExternalInput").ap()
    msk_d = nc.dram_tensor("msk", [P, t_tiles * B], F32R, kind="ExternalInput").ap()
    out_d = nc.dram_tensor("out", [B, H], F32, kind="ExternalOutput").ap()

    with TileKernel(nc) as (tc, ctx):
        const = ctx.enter_context(tc.tile_pool(name="const", bufs=1))
        xpool = ctx.enter_context(tc.tile_pool(name="x", bufs=x_bufs))
        hpool = ctx.enter_context(tc.tile_pool(name="h", bufs=3))
        htpool = ctx.enter_context(tc.tile_pool(name="ht", bufs=2))
        h1pool = ctx.enter_context(tc.tile_pool(name="h1", bufs=2))
        h2pool = ctx.enter_context(tc.tile_pool(name="h2", bufs=2))
        pmix0 = ctx.enter_context(tc.tile_pool(name="pmix0", bufs=2, space="PSUM"))
        pmix1 = ctx.enter_context(tc.tile_pool(name="pmix1", bufs=2, space="PSUM"))
        pshared = ctx.enter_context(tc.tile_pool(name="pshared", bufs=3, space="PSUM"))
        ptr = pmm1 = pmm2 = pshared
        ppool = ctx.enter_context(tc.tile_pool(name="ppool", bufs=1, space="PSUM"))

        # ---- constants into SBUF via SWDGE (gpsimd), keeping the HWDGE
        # rings free for the x stream ----
        seye = const.tile([P, n_layers * P], F32R)
        nc.gpsimd.dma_start(seye[:], seye_d[:])
        svec = const.tile([P, n_layers], F32)
        nc.gpsimd.dma_start(svec[:], svec_d[:])
        ident = const.tile([P, P], F32)
        nc.gpsimd.dma_start(ident[:], ident_d[:])
        w1t = const.tile([P, FC * H], F32R)
        nc.gpsimd.dma_start(w1t[:], w1t_d[:])
        w2t = const.tile([P, HC * H], F32R)
        nc.gpsimd.dma_start(w2t[:], w2t_d[:])
        b1 = const.tile([P, HC], F32)
        nc.gpsimd.dma_start(b1[:], b1_d[:])
        b2rep = const.tile([P, H], F32R)
        nc.gpsimd.dma_start(b2rep[:], b2rep_d[:])
        ones = const.tile([P, P], F32R)
        nc.gpsimd.dma_start(ones[:], ones_d[:])
        msk = const.tile([P, t_tiles * B], F32R)
        nc.gpsimd.dma_start(msk[:], msk_d[:])

        pooled_sb = const.tile([B, H], F32)

        # MLP chunks of up to 2 token tiles (moving dim 256)
        chunk_plan = []
        t = 0
        while t < t_tiles:
            n_t = min(2, t_tiles - t)
            chunk_plan.append((t, n_t))
            t += n_t

        if isinstance(dma_pieces, (list, tuple)):
            bounds = sorted({min(bd, n_layers) for bd in dma_pieces} | {0, n_layers})
        else:
            bounds = [round(i * n_layers / dma_pieces)
                      for i in range(dma_pieces + 1)]

        def mlp_chunk(hT, t0, n_t, ppooled):
            cw = n_t * P
            h1 = h1pool.tile([P, HC * CWMAX], F32R, tag="h1")
            for m in range(HC):
                o1 = pmm1.tile([P, CWMAX], F32, tag="po")
                for k in range(FC):
                    nc.tensor.matmul(o1[:, 0:cw],
                                     w1t[:, k * H + m * P: k * H + (m + 1) * P],
                                     hT[:, k * CWMAX: k * CWMAX + cw],
                                     start=(k == 0), stop=(k == FC - 1))
                nc.scalar.activation(h1[:, m * CWMAX: m * CWMAX + cw],
                                     o1[:, 0:cw],
                                     mybir.ActivationFunctionType.Relu,
                                     bias=b1[:, m:m + 1], scale=1.0)
            for s in range(n_t):
                gi = t0 + s
                o2 = pmm2.tile([P, H], F32, tag="po")
                # bias enters the accumulation: ones.T @ (b2/128) == +b2 row
                nc.tensor.matmul(o2[:], ones[:], b2rep[:],
                                 start=True, stop=False)
                for m in range(HC):
                    nc.tensor.matmul(o2[:],
                                     h1[:, m * CWMAX + s * P: m * CWMAX + (s + 1) * P],
                                     w2t[:, m * H:(m + 1) * H],
                                     start=False, stop=(m == HC - 1))
                h2 = h2pool.tile([P, H], F32R, tag="h2")
                nc.scalar.activation(h2[:], o2[:],
                                     mybir.ActivationFunctionType.Relu)
                # per-sample masked pooling: msk tile is [token, 16] 0/1
                nc.tensor.matmul(ppooled[:], msk[:, gi * B:(gi + 1) * B], h2[:],
                                 start=(gi == 0), stop=(gi == t_tiles - 1),
                                 skip_group_check=True)

        def _body(_iv=None):
            ppooled = ppool.tile([B, H], F32, tag="pool")
            for (t0, n_t) in chunk_plan:
                hT = htpool.tile([P, FC * CWMAX], F32R, tag="hT")
                for s in range(n_t):
                    ti = t0 + s
                    xt = xpool.tile([P, n_layers * F], F32R, tag="xt")
                    for lo, hi in zip(bounds[:-1], bounds[1:]):
                        nc.sync.dma_start(
                            xt[:, lo * F:hi * F],
                            xp_d[ti * P:(ti + 1) * P, lo * F:hi * F])
                    pm0 = pmix0.tile([P, 512], F32, tag="pm0")
                    pm1 = pmix1.tile([P, F - 512], F32, tag="pm1")
                    accd = None
                    for l in range(n_layers):
                        if l < n_pe_layers:
                            se = seye[:, l * P:(l + 1) * P]
                            st, sp = (l == 0), (l == n_pe_layers - 1)
                            nc.tensor.matmul(pm0[:], se, xt[:, l * F: l * F + 512],
                                             start=st, stop=sp)
                            nc.tensor.matmul(pm1[:], se, xt[:, l * F + 512:(l + 1) * F],
                                             start=st, stop=sp)
                        else:
                            xf = xt[:, l * F:(l + 1) * F].bitcast(F32)
                            sc = svec[:, l:l + 1]
                            if accd is None:
                                accd = hpool.tile([P, F], F32, tag="accd")
                                nc.vector.tensor_scalar_mul(accd[:], xf, sc)
                            else:
                                nc.vector.scalar_tensor_tensor(
                                    accd[:], xf, sc, accd[:],
                                    op0=mybir.AluOpType.mult,
                                    op1=mybir.AluOpType.add)
                    # PSUM (+ DVE partial) -> SBUF mixed tile
                    h = hpool.tile([P, F], F32, tag="h")
                    if accd is None:
                        nc.scalar.copy(h[:, 0:512], pm0[:])
                        nc.scalar.copy(h[:, 512:F], pm1[:])
                    else:
                        nc.vector.scalar_tensor_tensor(
                            h[:, 0:512], pm0[:], 1.0, accd[:, 0:512],
                            op0=mybir.AluOpType.bypass, op1=mybir.AluOpType.add)
                        nc.vector.scalar_tensor_tensor(
                            h[:, 512:F], pm1[:], 1.0, accd[:, 512:F],
                            op0=mybir.AluOpType.bypass, op1=mybir.AluOpType.add)
                    # transpose 128x128 blocks into hT
                    for fc in range(FC):
                        pt = ptr.tile([P, P], F32, tag="po")
                        nc.tensor.transpose(pt[:], h[:, fc * P:(fc + 1) * P], ident[:])
                        dst = hT[:, fc * CWMAX + s * P: fc * CWMAX + (s + 1) * P]
                        if fc % 2 == 0:
                            nc.scalar.copy(dst, pt[:])
                        else:
                            nc.vector.tensor_copy(dst, pt[:])
                mlp_chunk(hT, t0, n_t, ppooled)
            nc.scalar.copy(pooled_sb[:], ppooled[:])

        if hw_loop_repeat is not None and hw_loop_repeat > 1:
            with tc.For_i(0, hw_loop_repeat, 1) as _i:
                _body(_i)
        else:
            _body()

        nc.sync.dma_start(out_d[:], pooled_sb[:])

    if split_waits:
        _split_excess_waits(nc, max_waits=1)
    return nc


class TileKernel:
    """TileContext + ExitStack in one `with`."""

    def __init__(self, nc):
        self.tc = tile.TileContext(nc)
        self.ctx = ExitStack()

    def __enter__(self):
        tc = self.tc.__enter__()
        self.ctx.__enter__()
        return tc, self.ctx

    def __exit__(self, *exc):
        self.ctx.__exit__(*exc)
        return self.tc.__exit__(*exc)


_PROGRAM_CACHE: dict[tuple, bass.Bass] = {}


def _get_program(n_layers: int, t_tiles: int) -> bass.Bass:
    key = (n_layers, t_tiles)
    if key not in _PROGRAM_CACHE:
        _PROGRAM_CACHE[key] = build_program(n_layers, t_tiles)
    return _PROGRAM_CACHE[key]


def _softmax32(v: np.ndarray) -> np.ndarray:
    v = v.astype(np.float32)
    e = np.exp(v - v.max())
    return (e / e.sum()).astype(np.float32)


def _prep_in_maps(inputs: dict, n_layers: int):
    x = np.asarray(inputs["x"])
    lengths = np.asarray(inputs["lengths"]).astype(np.int64)

    t_tiles, bs, ts, val = _plan_packing(lengths)
    cap = t_tiles * P

    # host-side prep of the small replicated operands
    s = (_softmax32(np.asarray(inputs["mixing_weights"]))
         * np.float32(np.asarray(inputs["gamma"]).reshape(-1)[0]))
    seye = np.zeros((P, n_layers * P), np.float32)
    for l in range(n_layers):
        seye[:, l * P:(l + 1) * P] = np.eye(P, dtype=np.float32) * s[l]
    svec = np.tile(s[:n_layers], (P, 1)).astype(np.float32)
    ident = np.eye(P, dtype=np.float32)

    W1 = np.asarray(inputs["W1"], np.float32)  # [H, F]
    W2 = np.asarray(inputs["W2"], np.float32)  # [H, H]
    w1t = np.ascontiguousarray(
        W1.T.reshape(FC, P, H).transpose(1, 0, 2).reshape(P, FC * H))
    w2t = np.ascontiguousarray(
        W2.T.reshape(HC, P, H).transpose(1, 0, 2).reshape(P, HC * H))
    b1p = np.ascontiguousarray(np.asarray(inputs["b1"], np.float32).reshape(HC, P).T)
    b2rep = np.tile(np.asarray(inputs["b2"], np.float32).reshape(1, H) / P, (P, 1))
    onesm = np.ones((P, P), np.float32)

    in_maps = []
    for c in range(N_CORES):
        xp = np.ascontiguousarray(
            x[bs[c], :n_layers, ts[c], :].reshape(cap, n_layers * F))
        mm = np.zeros((cap, B), np.float32)
        mm[np.arange(cap), bs[c]] = val[c]
        mskp = np.ascontiguousarray(
            mm.reshape(t_tiles, P, B).transpose(1, 0, 2).reshape(P, t_tiles * B))
        in_maps.append({
            "xp": xp, "seye": seye, "svec": svec, "ident": ident,
            "w1t": w1t, "w2t": w2t, "b1": b1p, "b2rep": b2rep,
            "ones": onesm, "msk": mskp,
        })
    return in_maps, t_tiles


def _finish(pooled_parts, inputs):
    lengths = np.asarray(inputs["lengths"]).astype(np.float32)
    Wl = np.asarray(inputs["Wl"], np.float32)
    bl = np.asarray(inputs["bl"], np.float32)
    pooled = np.sum(np.stack(pooled_parts, 0), axis=0, dtype=np.float32)
    pooled = pooled / lengths[:, None]
    return (pooled @ Wl.T + bl).astype(np.float32)


def kernel(x, lengths, layer, gamma, mixing_weights, W1, b1, W2, b2, Wl, bl):
    n_layers = int(np.asarray(layer)) + 1
    assert 1 <= n_layers <= L

    inputs = dict(x=x, lengths=lengths, gamma=gamma,
                  mixing_weights=mixing_weights,
                  W1=W1, b1=b1, W2=W2, b2=b2, Wl=Wl, bl=bl)
    in_maps, t_tiles = _prep_in_maps(inputs, n_layers)
    nc = _get_program(n_layers, t_tiles)

    res = run_bass_kernel_spmd(nc, in_maps, list(range(N_CORES)))
    return _finish([res.results[c]["out"] for c in range(N_CORES)], inputs)


# revision 12
# speedup vs baseline: 1.9147x; 1.0207x over previous
"""Trainium2 Bass kernel for nn_CumulativeProbingDense.

Computation (see reference):
    h      = sum_l softmax(mixing_weights)[l] * x[:, l] * gamma   # [B, S, F]
    h1     = relu(h @ W1.T + b1)                                  # [B, S, H]
    h2     = relu(h1 @ W2.T + b2)                                 # [B, S, H]
    pooled = (h2 * mask).sum(S) / lengths                         # [B, H]
    logits = pooled @ Wl.T + bl                                   # [B, NL]

Sharding: tokens masked out by `lengths` (t >= lengths[b]) cannot affect the
output, so only the sum(lengths) valid tokens are processed.  The host packs
the valid (sample, token) pairs of ALL samples into one stream, split evenly
across the 8 cores (token-balanced data parallel).  Each core streams its
packed x slice ([t_tiles*128 tokens, L, F], token-major so DMA runs are
contiguous), computes the layer mix + MLP, and mask-matmul-pools per sample.
Host combines the per-core pooled partials and applies the tiny final linear.

Device pipeline per 128-token tile:
  - layer mix on TensorE: PSUM-accumulated matmuls with scaled-identity
    stationary; optionally a few trailing layers accumulate on the DVE
  - PE-transpose mixed tile into hT [feat part, token free]
  - mm1 (W1T stationary) -> relu+b1 on ScalarE -> h1 [hid part, token free]
  - mm2 with h1 chunks as STATIONARY and W2T moving -> h2 [token part, hid
    free]; b2 enters the same PSUM group as a ones x (b2/128) matmul
  - pooling: matmul with the per-sample 0/1 mask [token, 16] stationary,
    PSUM-accumulated over all tiles -> pooled [16, 256] per core
"""

import numpy as np

import concourse.bass as bass
import concourse.tile as tile
from concourse import mybir
from concourse.bass_utils import run_bass_kernel_spmd
from contextlib import ExitStack

F32 = mybir.dt.float32
F32R = mybir.dt.float32r

N_CORES = 8
B, L, S, F = 16, 13, 1024, 768
H, NL = 256, 7
P = 128                       # SBUF partitions
FC = F // P                   # feature chunks of 128
HC = H // P                   # hidden chunks of 128
CWMAX = 2 * P                 # token width of one MLP chunk

# matmul input dtype: float32r streams at 1 cycle/row (moving dim >= 256)
# vs plain float32's 4 cycles/row. fp32r rounds the operands (TF32-like).
MM_DT = F32R


def _split_excess_waits(nc, max_waits=1):
    """walrus (CoreV3) rejects instructions carrying more than a couple of
    sync waits (e.g. the TileContext exit drain). Hoist excess waits onto
    standalone NoOps inserted before the offending instruction."""
    n_fixed = 0
    for f in nc.m.functions:
        for bb in f.blocks:
            out, changed = [], False
            for inst in bb.instructions:
                si = getattr(inst, "sync_info", None)
                if si is not None and len(si.on_wait) > max_waits:
                    waits = list(si.on_wait)
                    for j, w in enumerate(waits[max_waits:]):
                        out.append(mybir.InstNoOp(
                            name=f"{inst.name}-wsplit{j}",
                            engine=inst.engine, ins=[], outs=[],
                            sync_info=mybir.SyncInfo(on_wait=[w], on_update=[]),
                        ))
                    inst.sync_info = mybir.SyncInfo(
                        on_wait=waits[:max_waits], on_update=list(si.on_update))
                    changed = True
                    n_fixed += 1
                out.append(inst)
            if changed:
                bb.instructions = out
    return n_fixed


def _plan_packing(lengths):
    """Token-balanced packing of all valid (sample, token) pairs onto cores.

    Returns (t_tiles, last_tw, b_idx [n_cores, cap], t_idx [n_cores, cap],
    valid [n_cores, cap]) with cap = (t_tiles-1)*128 + last_tw slots per
    core (the final token tile is partial); pad slots point at (0, 0) with
    valid=0."""
    lengths = np.asarray(lengths).astype(np.int64)
    total = int(lengths.sum())
    cap = max(1, -(-total // N_CORES))
    t_tiles = -(-cap // P)
    last_tw = cap - (t_tiles - 1) * P
    bs = np.repeat(np.arange(B, dtype=np.int64), lengths)
    ts = np.concatenate([np.arange(n, dtype=np.int64) for n in lengths])
    pad = N_CORES * cap - total
    bs = np.concatenate([bs, np.zeros(pad, np.int64)])
    ts = np.concatenate([ts, np.zeros(pad, np.int64)])
    val = np.concatenate([np.ones(total, np.float32), np.zeros(pad, np.float32)])
    return (t_tiles, last_tw, bs.reshape(N_CORES, cap),
            ts.reshape(N_CORES, cap), val.reshape(N_CORES, cap))


def build_program(n_layers: int, t_tiles: int, last_tw: int = P,
                  split_waits: bool = True,
                  hw_loop_repeat: int | None = None,
                  mix_dve_layers: int = 3,
                  x_bufs: int = 3,
                  dma_pieces=(11,)) -> bass.Bass:
    # mix_dve_layers: trailing layers accumulated on the DVE (axpy) instead
    # of the TensorE, to balance PE vs DVE occupancy.
    n_pe_layers = n_layers - min(mix_dve_layers, n_layers - 1)
    cap = (t_tiles - 1) * P + last_tw
    nc = bass.Bass("TRN2", target_bir_lowering=False, debug=False, num_devices=1)

    xp_d = nc.dram_tensor("xp", [cap, n_layers * F], F32R, kind="ExternalInput").ap()
    seye_d = nc.dram_tensor("seye", [P, n_layers * P], F32R, kind="ExternalInput").ap()
    svec_d = nc.dram_tensor("svec", [P, n_layers], F32, kind="ExternalInput").ap()
    ident_d = nc.dram_tensor("ident", [P, P], F32, kind="ExternalInput").ap()
    w1t_d = nc.dram_tensor("w1t", [P, FC * H], F32R, kind="ExternalInput").ap()
    w2t_d = nc.dram_tensor("w2t", [P, HC * H], F32R, kind="ExternalInput").ap()
    b1_d = nc.dram_tensor("b1", [P, HC], F32, kind="ExternalInput").ap()
    b2rep_d = nc.dram_tensor("b2rep", [P, H], F32R, kind="ExternalInput").ap()
    ones_d = nc.dram_tensor("ones", [P, P], F32R, kind="ExternalInput").ap()
    msk_d = nc.dram_tensor("msk", [P, t_tiles * B], F32R, kind="ExternalInput").ap()
    out_d = nc.dram_tensor("out", [B, H], F32, kind="ExternalOutput").ap()

    with TileKernel(nc) as (tc, ctx):
        const = ctx.enter_context(tc.tile_pool(name="const", bufs=1))
        xpool = ctx.enter_context(tc.tile_pool(name="x", bufs=x_bufs))
        hpool = ctx.enter_context(tc.tile_pool(name="h", bufs=3))
        htpool = ctx.enter_context(tc.tile_pool(name="ht", bufs=2))
        h1pool = ctx.enter_context(tc.tile_pool(name="h1", bufs=2))
        h2pool = ctx.enter_context(tc.tile_pool(name="h2", bufs=2))
        pmix0 = ctx.enter_context(tc.tile_pool(name="pmix0", bufs=2, space="PSUM"))
        pmix1 = ctx.enter_context(tc.tile_pool(name="pmix1", bufs=2, space="PSUM"))
        pshared = ctx.enter_context(tc.tile_pool(name="pshared", bufs=3, space="PSUM"))
        ptr = pmm1 = pmm2 = pshared
        ppool = ctx.enter_context(tc.tile_pool(name="ppool", bufs=1, space="PSUM"))

        # ---- constants into SBUF via SWDGE (gpsimd), keeping the HWDGE
        # rings free for the x stream ----
        seye = const.tile([P, n_layers * P], F32R)
        nc.gpsimd.dma_start(seye[:], seye_d[:])
        svec = const.tile([P, n_layers], F32)
        nc.gpsimd.dma_start(svec[:], svec_d[:])
        ident = const.tile([P, P], F32)
        nc.gpsimd.dma_start(ident[:], ident_d[:])
        w1t = const.tile([P, FC * H], F32R)
        nc.gpsimd.dma_start(w1t[:], w1t_d[:])
        w2t = const.tile([P, HC * H], F32R)
        nc.gpsimd.dma_start(w2t[:], w2t_d[:])
        b1 = const.tile([P, HC], F32)
        nc.gpsimd.dma_start(b1[:], b1_d[:])
        b2rep = const.tile([P, H], F32R)
        nc.gpsimd.dma_start(b2rep[:], b2rep_d[:])
        ones = const.tile([P, P], F32R)
        nc.gpsimd.dma_start(ones[:], ones_d[:])
        msk = const.tile([P, t_tiles * B], F32R)
        nc.gpsimd.dma_start(msk[:], msk_d[:])

        pooled_sb = const.tile([B, H], F32)

        # MLP chunks of up to 2 token tiles (moving dim 256); the final
        # tile gets its own chunk so the post-DMA tail chain stays short
        chunk_plan = []
        rem = t_tiles - 1
        t = 0
        while t + 1 < rem:
            chunk_plan.append((t, 2))
            t += 2
        if t < rem:
            chunk_plan.append((t, 1))
            t += 1
        chunk_plan.append((t_tiles - 1, 1))

        if isinstance(dma_pieces, (list, tuple)):
            bounds = sorted({min(bd, n_layers) for bd in dma_pieces} | {0, n_layers})
        else:
            bounds = [round(i * n_layers / dma_pieces)
                      for i in range(dma_pieces + 1)]

        def mlp_chunk(hT, t0, n_t, ppooled):
            cw = n_t * P
            h1 = h1pool.tile([P, HC * CWMAX], F32R, tag="h1")
            for m in range(HC):
                o1 = pmm1.tile([P, CWMAX], F32, tag="po")
                for k in range(FC):
                    nc.tensor.matmul(o1[:, 0:cw],
                                     w1t[:, k * H + m * P: k * H + (m + 1) * P],
                                     hT[:, k * CWMAX: k * CWMAX + cw],
                                     start=(k == 0), stop=(k == FC - 1))
                nc.scalar.activation(h1[:, m * CWMAX: m * CWMAX + cw],
                                     o1[:, 0:cw],
                                     mybir.ActivationFunctionType.Relu,
                                     bias=b1[:, m:m + 1], scale=1.0)
            for s in range(n_t):
                gi = t0 + s
                o2 = pmm2.tile([P, H], F32, tag="po")
                # bias enters the accumulation: ones.T @ (b2/128) == +b2 row
                nc.tensor.matmul(o2[:], ones[:], b2rep[:],
                                 start=True, stop=False)
                for m in range(HC):
                    nc.tensor.matmul(o2[:],
                                     h1[:, m * CWMAX + s * P: m * CWMAX + (s + 1) * P],
                                     w2t[:, m * H:(m + 1) * H],
                                     start=False, stop=(m == HC - 1))
                h2 = h2pool.tile([P, H], F32R, tag="h2")
                nc.scalar.activation(h2[:], o2[:],
                                     mybir.ActivationFunctionType.Relu)
                # per-sample masked pooling: msk tile is [token, 16] 0/1
                nc.tensor.matmul(ppooled[:], msk[:, gi * B:(gi + 1) * B], h2[:],
                                 start=(gi == 0), stop=(gi == t_tiles - 1),
                                 skip_group_check=True)

        def _body(_iv=None):
            ppooled = ppool.tile([B, H], F32, tag="pool")
            for (t0, n_t) in chunk_plan:
                hT = htpool.tile([P, FC * CWMAX], F32R, tag="hT")
                for s in range(n_t):
                    ti = t0 + s
                    tw = last_tw if ti == t_tiles - 1 else P
                    # the partial final tile runs an all-PE mix: PSUM rows
                    # >= tw come out zero, so no stale SBUF is ever read
                    n_pe = n_layers if tw < P else n_pe_layers
                    xt = xpool.tile([P, n_layers * F], F32R, tag="xt")
                    for lo, hi in zip(bounds[:-1], bounds[1:]):
                        nc.sync.dma_start(
                            xt[0:tw, lo * F:hi * F],
                            xp_d[ti * P: ti * P + tw, lo * F:hi * F])
                    pm0 = pmix0.tile([P, 512], F32, tag="pm0")
                    pm1 = pmix1.tile([P, F - 512], F32, tag="pm1")
                    accd = None
                    for l in range(n_layers):
                        if l < n_pe:
                            se = seye[0:tw, l * P:(l + 1) * P]
                            st, sp = (l == 0), (l == n_pe - 1)
                            nc.tensor.matmul(pm0[:], se, xt[0:tw, l * F: l * F + 512],
                                             start=st, stop=sp)
                            nc.tensor.matmul(pm1[:], se, xt[0:tw, l * F + 512:(l + 1) * F],
                                             start=st, stop=sp)
                        else:
                            xf = xt[:, l * F:(l + 1) * F].bitcast(F32)
                            sc = svec[:, l:l + 1]
                            if accd is None:
                                accd = hpool.tile([P, F], F32, tag="accd")
                                nc.vector.tensor_scalar_mul(accd[:], xf, sc)
                            else:
                                nc.vector.scalar_tensor_tensor(
                                    accd[:], xf, sc, accd[:],
                                    op0=mybir.AluOpType.mult,
                                    op1=mybir.AluOpType.add)
                    # PSUM (+ DVE partial) -> SBUF mixed tile
                    h = hpool.tile([P, F], F32, tag="h")
                    if accd is None:
                        nc.scalar.copy(h[:, 0:512], pm0[:])
                        nc.scalar.copy(h[:, 512:F], pm1[:])
                    else:
                        nc.vector.scalar_tensor_tensor(
                            h[:, 0:512], pm0[:], 1.0, accd[:, 0:512],
                            op0=mybir.AluOpType.bypass, op1=mybir.AluOpType.add)
                        nc.vector.scalar_tensor_tensor(
                            h[:, 512:F], pm1[:], 1.0, accd[:, 512:F],
                            op0=mybir.AluOpType.bypass, op1=mybir.AluOpType.add)
                    # transpose 128x128 blocks into hT
                    for fc in range(FC):
                        pt = ptr.tile([P, P], F32, tag="po")
                        nc.tensor.transpose(pt[:], h[:, fc * P:(fc + 1) * P], ident[:])
                        dst = hT[:, fc * CWMAX + s * P: fc * CWMAX + (s + 1) * P]
                        if fc % 2 == 0:
                            nc.scalar.copy(dst, pt[:])
                        else:
                            nc.vector.tensor_copy(dst, pt[:])
                mlp_chunk(hT, t0, n_t, ppooled)
            nc.scalar.copy(pooled_sb[:], ppooled[:])

        if hw_loop_repeat is not None and hw_loop_repeat > 1:
            with tc.For_i(0, hw_loop_repeat, 1) as _i:
                _body(_i)
        else:
            _body()

        nc.sync.dma_start(out_d[:], pooled_sb[:])

    if split_waits:
        _split_excess_waits(nc, max_waits=1)
    return nc


class TileKernel:
    """TileContext + ExitStack in one `with`."""

    def __init__(self, nc):
        self.tc = tile.TileContext(nc)
        self.ctx = ExitStack()

    def __enter__(self):
        tc = self.tc.__enter__()
        self.ctx.__enter__()
        return tc, self.ctx

    def __exit__(self, *exc):
        self.ctx.__exit__(*exc)
        return self.tc.__exit__(*exc)


_PROGRAM_CACHE: dict[tuple, bass.Bass] = {}


def _get_program(n_layers: int, t_tiles: int, last_tw: int) -> bass.Bass:
    key = (n_layers, t_tiles, last_tw)
    if key not in _PROGRAM_CACHE:
        _PROGRAM_CACHE[key] = build_program(n_layers, t_tiles, last_tw)
    return _PROGRAM_CACHE[key]


def _softmax32(v: np.ndarray) -> np.ndarray:
    v = v.astype(np.float32)
    e = np.exp(v - v.max())
    return (e / e.sum()).astype(np.float32)


def _prep_in_maps(inputs: dict, n_layers: int):
    x = np.asarray(inputs["x"])
    lengths = np.asarray(inputs["lengths"]).astype(np.int64)

    t_tiles, last_tw, bs, ts, val = _plan_packing(lengths)
    cap = (t_tiles - 1) * P + last_tw

    # host-side prep of the small replicated operands
    s = (_softmax32(np.asarray(inputs["mixing_weights"]))
         * np.float32(np.asarray(inputs["gamma"]).reshape(-1)[0]))
    seye = np.zeros((P, n_layers * P), np.float32)
    for l in range(n_layers):
        seye[:, l * P:(l + 1) * P] = np.eye(P, dtype=np.float32) * s[l]
    svec = np.tile(s[:n_layers], (P, 1)).astype(np.float32)
    ident = np.eye(P, dtype=np.float32)

    W1 = np.asarray(inputs["W1"], np.float32)  # [H, F]
    W2 = np.asarray(inputs["W2"], np.float32)  # [H, H]
    w1t = np.ascontiguousarray(
        W1.T.reshape(FC, P, H).transpose(1, 0, 2).reshape(P, FC * H))
    w2t = np.ascontiguousarray(
        W2.T.reshape(HC, P, H).transpose(1, 0, 2).reshape(P, HC * H))
    b1p = np.ascontiguousarray(np.asarray(inputs["b1"], np.float32).reshape(HC, P).T)
    b2rep = np.tile(np.asarray(inputs["b2"], np.float32).reshape(1, H) / P, (P, 1))
    onesm = np.ones((P, P), np.float32)

    in_maps = []
    for c in range(N_CORES):
        xp = np.ascontiguousarray(
            x[bs[c], :n_layers, ts[c], :].reshape(cap, n_layers * F))
        mm = np.zeros((t_tiles * P, B), np.float32)
        mm[np.arange(cap), bs[c]] = val[c]
        mskp = np.ascontiguousarray(
            mm.reshape(t_tiles, P, B).transpose(1, 0, 2).reshape(P, t_tiles * B))
        in_maps.append({
            "xp": xp, "seye": seye, "svec": svec, "ident": ident,
            "w1t": w1t, "w2t": w2t, "b1": b1p, "b2rep": b2rep,
            "ones": onesm, "msk": mskp,
        })
    return in_maps, dict(t_tiles=t_tiles, last_tw=last_tw)


def _finish(pooled_parts, inputs):
    lengths = np.asarray(inputs["lengths"]).astype(np.float32)
    Wl = np.asarray(inputs["Wl"], np.float32)
    bl = np.asarray(inputs["bl"], np.float32)
    pooled = np.sum(np.stack(pooled_parts, 0), axis=0, dtype=np.float32)
    pooled = pooled / lengths[:, None]
    return (pooled @ Wl.T + bl).astype(np.float32)


def kernel(x, lengths, layer, gamma, mixing_weights, W1, b1, W2, b2, Wl, bl):
    n_layers = int(np.asarray(layer)) + 1
    assert 1 <= n_layers <= L

    inputs = dict(x=x, lengths=lengths, gamma=gamma,
                  mixing_weights=mixing_weights,
                  W1=W1, b1=b1, W2=W2, b2=b2, Wl=Wl, bl=bl)
    in_maps, pa = _prep_in_maps(inputs, n_layers)
    nc = _get_program(n_layers, pa["t_tiles"], pa["last_tw"])

    res = run_bass_kernel_spmd(nc, in_maps, list(range(N_CORES)))
    return _finish([res.results[c]["out"] for c in range(N_CORES)], inputs)
